# revision 7
# baseline (speedup 1.0000x reference)
"""GQA attention (B=1, T=2048, D=2048, H=32, KVH=8, HD=64) on 8 TRN2 cores.

Head-tensor-parallel: core c owns kv-head c and q-heads 4c..4c+3.
wq/wk/wv column-parallel, wo row-parallel; partials summed on host.
"""
import sys

if "/opt/trn_rl_repo" not in sys.path:
    sys.path.insert(0, "/opt/trn_rl_repo")

import numpy as np
import ml_dtypes

import concourse.bacc as bacc
import concourse.mybir as mybir
import concourse.tile as tile
from concourse.bass_utils import run_bass_kernel_spmd

BF16 = ml_dtypes.bfloat16
T, D, H, KVH, HD = 2048, 2048, 32, 8, 64
NCORES = 8
HPC = H // NCORES            # 4 q heads per core
KT, PT = 16, 128             # k-tiles of 128 over D
NCH = 4                      # t chunks of 512
CH = 512

_cache = {}


def _build_nc():
    if "nc" in _cache:
        return _cache["nc"]
    fp32, bf16 = mybir.dt.float32, mybir.dt.bfloat16
    nc = bacc.Bacc("TRN2", target_bir_lowering=False, debug=False,
                   num_devices=NCORES)

    xt_d = nc.dram_tensor("xt", [D, T], bf16, kind="ExternalInput")
    wq_d = nc.dram_tensor("wq", [D, HPC * HD], bf16, kind="ExternalInput")
    wkv_d = nc.dram_tensor("wkv", [D, 2 * HD], bf16, kind="ExternalInput")
    wo_d = nc.dram_tensor("wo", [HPC * HD, D], bf16, kind="ExternalInput")
    cs4_d = nc.dram_tensor("cs4", [PT, T], bf16, kind="ExternalInput")
    sn4_d = nc.dram_tensor("sn4", [PT, T], bf16, kind="ExternalInput")
    pe_d = nc.dram_tensor("permE", [PT, 2 * PT], bf16, kind="ExternalInput")
    po_d = nc.dram_tensor("permO", [PT, 2 * PT], bf16, kind="ExternalInput")
    id_d = nc.dram_tensor("ident", [PT, PT], bf16, kind="ExternalInput")
    mk_d = nc.dram_tensor("masks", [PT, 4, CH], bf16, kind="ExternalInput")
    out_d = nc.dram_tensor("partial", [T, D], bf16, kind="ExternalOutput")

    with tile.TileContext(nc) as tc:
        with tc.tile_pool(name="const", bufs=1) as const, \
             tc.tile_pool(name="xtp", bufs=KT) as xtp, \
             tc.tile_pool(name="persist", bufs=1) as persist:

            # ---- loads -------------------------------------------------
            xt = []
            for k in range(KT):
                t_ = xtp.tile([PT, T], bf16, tag="xt")
                nc.sync.dma_start(t_[:], xt_d.ap()[k * PT:(k + 1) * PT, :])
                xt.append(t_)
            wq_sb = const.tile([PT, KT, HPC * HD], bf16, tag="wq")
            nc.sync.dma_start(wq_sb[:], wq_d.ap().rearrange("(k p) m -> p k m", p=PT))
            wkv_sb = const.tile([PT, KT, 2 * HD], bf16, tag="wkv")
            nc.sync.dma_start(wkv_sb[:], wkv_d.ap().rearrange("(k p) m -> p k m", p=PT))
            wo_sb = const.tile([PT, 2, D], bf16, tag="wo")
            nc.sync.dma_start(wo_sb[:], wo_d.ap().rearrange("(s p) m -> p s m", p=PT))
            cs4 = const.tile([PT, T], bf16, tag="cs4")
            nc.sync.dma_start(cs4[:], cs4_d.ap())
            sn4 = const.tile([PT, T], bf16, tag="sn4")
            nc.sync.dma_start(sn4[:], sn4_d.ap())
            permE = const.tile([PT, 2 * PT], bf16, tag="permE")
            nc.sync.dma_start(permE[:], pe_d.ap())
            permO = const.tile([PT, 2 * PT], bf16, tag="permO")
            nc.sync.dma_start(permO[:], po_d.ap())
            ident = const.tile([PT, PT], bf16, tag="ident")
            nc.sync.dma_start(ident[:], id_d.ap())
            masks = const.tile([PT, 4, CH], bf16, tag="masks")
            nc.sync.dma_start(masks[:], mk_d.ap())
            ones64 = const.tile([1, 64], bf16, tag="ones64")
            nc.vector.memset(ones64[:], 1.0)

            # persistent activations
            qt = [persist.tile([64, T], bf16, tag=f"qt{p}", name=f"qt{p}") for p in range(HPC)]
            kt = persist.tile([64, T], bf16, tag="kt")
            rE = persist.tile([PT, T], bf16, tag="rE")
            rO = persist.tile([PT, T], bf16, tag="rO")
            vx = [persist.tile([PT, HD + 1], bf16, tag=f"vx{s}", name=f"vx{s}") for s in range(KT)]
            ot = [persist.tile([PT, T], bf16, tag=f"ot{p}", name=f"ot{p}") for p in range(2)]

            # ---- phase 1: projections + rope + repack -------------------
            with tc.tile_pool(name="pp", bufs=4, space="PSUM") as pp, \
                 tc.tile_pool(name="rp", bufs=2, space="PSUM") as rp, \
                 tc.tile_pool(name="tmp", bufs=2) as tmp:
                for j in range(NCH):
                    jsl = slice(j * CH, (j + 1) * CH)
                    E = pp.tile([PT, CH], fp32, tag="pp")
                    O = pp.tile([PT, CH], fp32, tag="pp")
                    KV = pp.tile([PT, CH], fp32, tag="pp")
                    for k in range(KT):
                        st, sp = (k == 0), (k == KT - 1)
                        nc.tensor.matmul(E[:], wq_sb[:, k, 0:PT], xt[k][:, jsl],
                                         start=st, stop=sp)
                        nc.tensor.matmul(O[:], wq_sb[:, k, PT:2 * PT], xt[k][:, jsl],
                                         start=st, stop=sp)
                        nc.tensor.matmul(KV[:], wkv_sb[:, k, :], xt[k][:, jsl],
                                         start=st, stop=sp)
                    # rope q (full width, evens-major packing)
                    t1 = tmp.tile([PT, CH], fp32, tag="t1")
                    t2 = tmp.tile([PT, CH], fp32, tag="t2")
                    nc.vector.tensor_tensor(t1[:], E[:], cs4[:, jsl], mybir.AluOpType.mult)
                    nc.vector.tensor_tensor(t2[:], O[:], sn4[:, jsl], mybir.AluOpType.mult)
                    nc.vector.tensor_sub(rE[:, jsl], t1[:], t2[:])
                    t3 = tmp.tile([PT, CH], fp32, tag="t1")
                    t4 = tmp.tile([PT, CH], fp32, tag="t2")
                    nc.vector.tensor_tensor(t3[:], E[:], sn4[:, jsl], mybir.AluOpType.mult)
                    nc.vector.tensor_tensor(t4[:], O[:], cs4[:, jsl], mybir.AluOpType.mult)
                    nc.vector.tensor_add(rO[:, jsl], t3[:], t4[:])
                    # rope k (rows 0:32 evens, 32:64 odds of KV)
                    k1 = tmp.tile([32, CH], fp32, tag="k1")
                    k2 = tmp.tile([32, CH], fp32, tag="k2")
                    nc.vector.tensor_tensor(k1[:], KV[0:32, :], cs4[0:32, jsl], mybir.AluOpType.mult)
                    nc.vector.tensor_tensor(k2[:], KV[32:64, :], sn4[0:32, jsl], mybir.AluOpType.mult)
                    nc.vector.tensor_sub(kt[0:32, jsl], k1[:], k2[:])
                    k3 = tmp.tile([32, CH], fp32, tag="k1")
                    k4 = tmp.tile([32, CH], fp32, tag="k2")
                    nc.vector.tensor_tensor(k3[:], KV[0:32, :], sn4[0:32, jsl], mybir.AluOpType.mult)
                    nc.vector.tensor_tensor(k4[:], KV[32:64, :], cs4[0:32, jsl], mybir.AluOpType.mult)
                    nc.vector.tensor_add(kt[32:64, jsl], k3[:], k4[:])
                    # v -> natural layout tiles (+ ones column)
                    vt = tmp.tile([64, CH], bf16, tag="vt")
                    nc.vector.tensor_copy(vt[:], KV[64:PT, :])
                    for u in range(4):
                        s_idx = 4 * j + u
                        vtr = rp.tile([PT, 64], bf16, tag="vtr")
                        nc.tensor.transpose(vtr[:], vt[:, u * PT:(u + 1) * PT],
                                            ident[:64, :64])
                        nc.vector.tensor_copy(vx[s_idx][:, 0:HD], vtr[:])
                        nc.vector.memset(vx[s_idx][:, HD:HD + 1], 1.0)
                    # repack q to head-contiguous layout (per head)
                    for p in range(HPC):
                        qp = rp.tile([64, CH], fp32, tag="qp")
                        nc.tensor.matmul(qp[:], permE[:, 64 * p:64 * p + 64],
                                         rE[:, jsl], start=True, stop=False)
                        nc.tensor.matmul(qp[:], permO[:, 64 * p:64 * p + 64],
                                         rO[:, jsl], start=False, stop=True)
                        nc.vector.tensor_copy(qt[p][:, jsl], qp[:])

            # ---- phase 2: attention ------------------------------------
            with tc.tile_pool(name="sc", bufs=2, space="PSUM") as scp, \
                 tc.tile_pool(name="pv", bufs=1, space="PSUM") as pvp, \
                 tc.tile_pool(name="ex", bufs=4) as exp_pool, \
                 tc.tile_pool(name="nrm", bufs=2) as nrm:
                for h in range(HPC):
                    q_h = qt[h]
                    pv = [pvp.tile([HD + 1, CH], fp32, tag=f"pv{j}", name=f"pv{h}_{j}") for j in range(NCH)]
                    for i in range(KT):
                        j0 = i // 4
                        ktsl = kt[:, i * PT:(i + 1) * PT]
                        # scores for chunk-pairs [0,1] and [2,3]
                        exs = {}
                        for g in range(2):
                            glo, ghi = 2 * g, 2 * g + 2
                            lo = max(j0, glo)
                            if lo >= ghi:
                                continue
                            sc = scp.tile([PT, 2 * CH], fp32, tag="sc")
                            for j in range(lo, ghi):
                                off = (j - glo) * CH
                                nc.tensor.matmul(sc[:, off:off + CH], ktsl,
                                                 q_h[:, j * CH:(j + 1) * CH],
                                                 start=True, stop=True)
                            ex = exp_pool.tile([PT, 2 * CH], bf16, tag="ex")
                            o0 = (lo - glo) * CH
                            nc.scalar.activation(ex[:, o0:(ghi - glo) * CH],
                                                 sc[:, o0:(ghi - glo) * CH],
                                                 mybir.ActivationFunctionType.Exp,
                                                 scale=0.125)
                            exs[g] = ex
                        # mask the diagonal chunk (j0) after exp
                        gd = j0 // 2
                        od = (j0 - 2 * gd) * CH
                        dsl = exs[gd][:, od:od + CH]
                        nc.gpsimd.tensor_tensor(dsl, dsl, masks[:, i % 4, :],
                                                mybir.AluOpType.mult)
                        for j in range(j0, NCH):
                            g = j // 2
                            off = (j - 2 * g) * CH
                            nc.tensor.matmul(pv[j][:], vx[i],
                                             exs[g][:, off:off + CH],
                                             start=(i == 0), stop=(i == 4 * j + 3))
                    for j in range(NCH):
                        srow = nrm.tile([1, CH], fp32, tag="srow")
                        nc.vector.tensor_copy(srow[:], pv[j][HD:HD + 1, :])
                        rrow = nrm.tile([1, CH], fp32, tag="rrow")
                        nc.vector.reciprocal_approx_fast(rrow[:], srow[:])
                        brow = nrm.tile([1, CH], bf16, tag="brow")
                        nc.vector.tensor_copy(brow[:], rrow[:])
                        bc = scp.tile([64, CH], fp32, tag="sc")
                        nc.tensor.matmul(bc[:], ones64[:], brow[:],
                                         start=True, stop=True)
                        bcs = nrm.tile([64, CH], fp32, tag="bcs")
                        nc.scalar.copy(bcs[:], bc[:])
                        nc.vector.tensor_tensor(
                            ot[h // 2][64 * (h % 2):64 * (h % 2) + 64, j * CH:(j + 1) * CH],
                            pv[j][0:HD, :], bcs[:], mybir.AluOpType.mult)

            # ---- phase 3: output projection ----------------------------
            with tc.tile_pool(name="wp", bufs=4, space="PSUM") as wpp, \
                 tc.tile_pool(name="po", bufs=4) as pop:
                n = 0
                for tt in range(KT):
                    for dd in range(NCH):
                        wp = wpp.tile([PT, CH], fp32, tag="wp")
                        for s in range(2):
                            nc.tensor.matmul(wp[:], ot[s][:, tt * PT:(tt + 1) * PT],
                                             wo_sb[:, s, dd * CH:(dd + 1) * CH],
                                             start=(s == 0), stop=(s == 1))
                        pout = pop.tile([PT, CH], bf16, tag="po")
                        if n % 2 == 0:
                            nc.scalar.copy(pout[:], wp[:])
                        else:
                            nc.vector.tensor_copy(pout[:], wp[:])
                        n += 1
                        nc.sync.dma_start(
                            out_d.ap()[tt * PT:(tt + 1) * PT, dd * CH:(dd + 1) * CH],
                            pout[:])

    nc.compile()
    _cache["nc"] = nc
    return nc


def _host_prep(x, freqs, wq, wk, wv, wo):
    x2d = np.asarray(x, np.float32)[0]                    # [T, D]
    xt = np.ascontiguousarray(x2d.T).astype(BF16)         # [D, T]
    cos = np.cos(np.asarray(freqs, np.float32))           # [T, 32]
    sin = np.sin(np.asarray(freqs, np.float32))
    cs4 = np.ascontiguousarray(np.tile(cos.T, (4, 1)), dtype=np.float32)  # [128, T]
    sn4 = np.ascontiguousarray(np.tile(sin.T, (4, 1)), dtype=np.float32)

    ev, od = np.arange(0, HD, 2), np.arange(1, HD, 2)

    # permE/permO [128, 256]: head h (cols 64h..64h+63): local row r<32 comes
    # from rE row 32h+r, r>=32 from rO row 32h+(r-32)
    permE = np.zeros((PT, 2 * PT), np.float32)
    permO = np.zeros((PT, 2 * PT), np.float32)
    for h in range(HPC):
        for r in range(32):
            permE[32 * h + r, 64 * h + r] = 1.0
            permO[32 * h + r, 64 * h + 32 + r] = 1.0

    ident = np.eye(PT, dtype=np.float32)

    masks = np.zeros((PT, 4, CH), np.float32)
    sig = np.arange(PT)[:, None]
    kap = np.arange(CH)[None, :]
    for r in range(4):
        masks[:, r, :] = (kap >= sig + PT * r).astype(np.float32)

    wq_f = np.asarray(wq, np.float32)
    wk_f = np.asarray(wk, np.float32)
    wv_f = np.asarray(wv, np.float32)
    wo_f = np.asarray(wo, np.float32)

    in_maps = []
    for c in range(NCORES):
        # wq for 4 heads, evens-major-across-heads packing:
        # cols 0:128 = [h0 evens, h1 evens, h2 evens, h3 evens], 128:256 odds
        blocks = [wq_f[:, (c * HPC + h) * HD:(c * HPC + h + 1) * HD] for h in range(HPC)]
        wq_c = np.concatenate([b[:, ev] for b in blocks] + [b[:, od] for b in blocks], axis=1)
        kblk = wk_f[:, c * HD:(c + 1) * HD]
        wkv_c = np.concatenate([kblk[:, ev], kblk[:, od],
                                wv_f[:, c * HD:(c + 1) * HD]], axis=1)
        wo_c = wo_f[c * HPC * HD:(c + 1) * HPC * HD, :]
        in_maps.append({
            "xt": xt,
            "wq": np.ascontiguousarray(wq_c).astype(BF16),
            "wkv": np.ascontiguousarray(wkv_c).astype(BF16),
            "wo": np.ascontiguousarray(wo_c).astype(BF16),
            "cs4": cs4.astype(BF16),
            "sn4": sn4.astype(BF16),
            "permE": permE.astype(BF16),
            "permO": permO.astype(BF16),
            "ident": ident.astype(BF16),
            "masks": masks.astype(BF16),
        })
    return in_maps


def run(inputs, trace=False, tmpdir=None):
    nc = _build_nc()
    in_maps = _host_prep(**inputs)
    res = run_bass_kernel_spmd(nc, in_maps, list(range(NCORES)),
                               trace=trace, tmpdir=tmpdir)
    acc = np.zeros((T, D), np.float32)
    for c in range(NCORES):
        acc += res.results[c]["partial"].astype(np.float32)
    return acc[None], res


def kernel(**inputs):
    out, _ = run(inputs, trace=False)
    return out
